# revision 1
# baseline (speedup 1.0000x reference)
"""Dual-head row-packed variant: head pairs share the 128-row PE array.

K^T and Q^T for heads (2hp, 2hp+1) are packed on partitions 0-63 / 64-127 of
the same SBUF tiles; the two 64-contraction score matmuls run concurrently in
different PE row-groups (tile_position auto-derived from base_partition) and
write different PSUM banks. Everything else matches kernel.py.
"""

import numpy as np
import ml_dtypes

B, H, S, DK = 4, 16, 2048, 64
NCORES = 8
HPC = H * B // NCORES
NPAIR = HPC // 2       # 4 head pairs
QT = 512
NQT = S // QT
KT = 128
NKT = S // KT
VE = DK + 1
SCALE = 1.0 / float(np.sqrt(DK))

_BF16 = ml_dtypes.bfloat16

_CACHE = {}


def _build_nc(reps=1):
    import concourse.mybir as mybir
    import concourse.tile as tile
    from concourse import bacc
    from concourse.masks import make_identity
    from contextlib import ExitStack

    dt = mybir.dt
    nc = bacc.Bacc()

    q2 = nc.declare_dram_parameter("q2", [NPAIR, 2 * DK, S], dt.bfloat16, isOutput=False)
    k2 = nc.declare_dram_parameter("k2", [NPAIR, 2 * DK, S], dt.bfloat16, isOutput=False)
    vex = nc.declare_dram_parameter("vex", [S, HPC, VE], dt.bfloat16, isOutput=False)
    maskT = nc.declare_dram_parameter("maskT", [S, S], dt.bfloat16, isOutput=False)
    out = nc.declare_dram_parameter("out", [HPC, S, DK], dt.float32, isOutput=True)

    with tile.TileContext(nc) as tc, ExitStack() as ctx:
        const = ctx.enter_context(tc.tile_pool(name="const", bufs=1))
        maskp = ctx.enter_context(tc.tile_pool(name="maskp", bufs=2))
        qp = ctx.enter_context(tc.tile_pool(name="qp", bufs=3))
        pp = ctx.enter_context(tc.tile_pool(name="pp", bufs=3))
        epi = ctx.enter_context(tc.tile_pool(name="epi", bufs=3))
        scps = ctx.enter_context(tc.tile_pool(name="scps", bufs=2, space="PSUM"))
        pvps = ctx.enter_context(tc.tile_pool(name="pvps", bufs=2, space="PSUM"))

        ident = const.tile([128, 128], dt.float32)
        make_identity(nc, ident)

        warm = const.tile([1, 2], dt.float32)
        nc.vector.memset(warm, 0.0)
        nc.scalar.activation(out=warm, in_=warm,
                             func=mybir.ActivationFunctionType.Exp)

        # K^T head pairs packed [128, pair, s]; pair 0 first for fast start
        k_sb = const.tile([2 * DK, NPAIR, S], dt.bfloat16)
        nc.sync.dma_start(out=k_sb[:, 0, :], in_=k2[0])
        nc.sync.dma_start(
            out=k_sb[:, 1:, :], in_=k2[1:].rearrange("h d s -> d h s")
        )

        v_sb = const.tile([KT, NKT, HPC, VE], dt.bfloat16)
        nc.sync.dma_start(out=v_sb, in_=vex.rearrange("(j p) h e -> p j h e", p=KT))

        for _rep in range(reps):
         for qt in range(NQT):
            m_sb = maskp.tile([KT, NKT, QT], dt.bfloat16)
            nc.sync.dma_start(
                out=m_sb,
                in_=maskT[:, qt * QT:(qt + 1) * QT].rearrange(
                    "(j p) q -> p j q", p=KT
                ),
            )
            for hp in range(NPAIR):
                q_sb = qp.tile([2 * DK, QT], dt.bfloat16)
                nc.sync.dma_start(out=q_sb, in_=q2[hp, :, qt * QT:(qt + 1) * QT])

                p_a = pp.tile([KT, NKT * QT], dt.bfloat16, tag="p_a")
                p_b = pp.tile([KT, NKT * QT], dt.bfloat16, tag="p_b")
                j0s = [0, 3, 6, 9, 12, 14]
                grps = [3, 3, 3, 3, 2, 2]
                for gi in range(6):
                    j0, grp = j0s[gi], grps[gi]
                    for a, p_sb in ((0, p_a), (1, p_b)):
                        sc = scps.tile([KT, 3 * QT], dt.float32, tag="sc")
                        for u in range(grp):
                            j = j0 + u
                            nc.tensor.matmul(
                                out=sc[:, u * QT:(u + 1) * QT],
                                lhsT=k_sb[64 * a:64 * a + 64, hp,
                                          j * KT:(j + 1) * KT],
                                rhs=q_sb[64 * a:64 * a + 64, :],
                                start=True,
                                stop=True,
                            )
                        nc.scalar.activation(
                            out=p_sb[:, j0 * QT:(j0 + grp) * QT],
                            in_=sc[:, 0:grp * QT],
                            func=mybir.ActivationFunctionType.Exp,
                            scale=SCALE,
                        )
                        nc.vector.tensor_mul(
                            p_sb[:, j0 * QT:(j0 + grp) * QT],
                            p_sb[:, j0 * QT:(j0 + grp) * QT],
                            m_sb[:, j0:j0 + grp, :].rearrange("p a q -> p (a q)"),
                        )

                for a, p_sb in ((0, p_a), (1, p_b)):
                    h = 2 * hp + a
                    pv = pvps.tile([128, QT], dt.float32, tag="pv")
                    for j in range(NKT):
                        nc.tensor.matmul(
                            out=pv[0:VE, :],
                            lhsT=v_sb[:, j, h, :],
                            rhs=p_sb[:, j * QT:(j + 1) * QT],
                            start=(j == 0),
                            stop=(j == NKT - 1),
                        )

                    o_sb = epi.tile([VE, QT], dt.float32, tag="o_sb")
                    nc.vector.tensor_copy(o_sb, pv[0:VE, :])

                    tr = pv[:, 0:4 * VE]
                    for j in range(4):
                        nc.tensor.transpose(
                            out=tr[:, j * VE:(j + 1) * VE],
                            in_=o_sb[:, j * 128:(j + 1) * 128],
                            identity=ident[0:VE, 0:VE],
                        )
                    ot = epi.tile([128, 4, VE], dt.float32, tag="ot")
                    nc.vector.tensor_copy(ot, tr.rearrange("p (a e) -> p a e", e=VE))

                    rec = epi.tile([128, 4], dt.float32, tag="rec")
                    nc.vector.reciprocal(rec, ot[:, :, DK])

                    outf = epi.tile([128, 4, DK], dt.float32, tag="outf")
                    for j in range(4):
                        nc.vector.tensor_scalar_mul(
                            outf[:, j, :], ot[:, j, 0:DK], rec[:, j:j + 1]
                        )
                    nc.sync.dma_start(
                        out=out[h, qt * QT:(qt + 1) * QT, :].rearrange(
                            "(j p) d -> p j d", p=128
                        ),
                        in_=outf,
                    )
    nc.compile()
    return nc


def _get_nc(reps=1):
    key = ("nc", reps)
    if key not in _CACHE:
        _CACHE[key] = _build_nc(reps)
    return _CACHE[key]


def _prep_core_inputs(q, k, v, m, core):
    b = core // (H // HPC)
    h0 = (core % (H // HPC)) * HPC
    qs = q[b, h0:h0 + HPC].transpose(0, 2, 1).astype(_BF16)   # [8, DK, S]
    ks = k[b, h0:h0 + HPC].transpose(0, 2, 1).astype(_BF16)
    q2 = np.empty((NPAIR, 2 * DK, S), dtype=_BF16)
    k2 = np.empty((NPAIR, 2 * DK, S), dtype=_BF16)
    for hp in range(NPAIR):
        q2[hp, :DK] = qs[2 * hp]
        q2[hp, DK:] = qs[2 * hp + 1]
        k2[hp, :DK] = ks[2 * hp]
        k2[hp, DK:] = ks[2 * hp + 1]
    vex = np.ones((S, HPC, VE), dtype=_BF16)
    vex[:, :, :DK] = v[b, h0:h0 + HPC].transpose(1, 0, 2)
    mT = m[b, 0].T.astype(_BF16)
    return {"q2": q2, "k2": k2, "vex": vex, "maskT": np.ascontiguousarray(mT)}


def kernel(query, key, value, mask):
    from concourse.bass_utils import run_bass_kernel_spmd

    q = np.asarray(query, dtype=np.float32)
    k = np.asarray(key, dtype=np.float32)
    v = np.asarray(value, dtype=np.float32)
    m = np.asarray(mask)

    nc = _get_nc()
    in_maps = [_prep_core_inputs(q, k, v, m, c) for c in range(NCORES)]
    res = run_bass_kernel_spmd(nc, in_maps, list(range(NCORES))).results

    out = np.empty((B, H, S, DK), dtype=np.float32)
    for c in range(NCORES):
        b = c // (H // HPC)
        h0 = (c % (H // HPC)) * HPC
        out[b, h0:h0 + HPC] = res[c]["out"]
    return out



# revision 2
# speedup vs baseline: 5.2593x; 5.2593x over previous
"""Act-saturated attention: the exp on the Activation engine is the hard
floor (~0.83ns/elem + ~275ns/instr). Design keeps Act 100% busy:

- scores: dual-head row-packed matmuls (two 64-contraction quadrant matmuls
  run concurrently), head0 -> 4-bank PSUM ring (acts of N=2048), head1 ->
  3-bank ring (acts of N=1536), interleaved so PE refills one ring while
  Act drains the other.
- mask applied post-exp on DVE (bf16 2x mode), epilogue division on DVE.
- PV is P-stationary: out[q,dk] with P^T tiles as stationary, V (+ones col)
  moving; 65-row matmuls measured at ~29ns. No output transposes needed.
- PV for head-pair hp runs software-pipelined during scores of hp+1.
"""

import numpy as np
import ml_dtypes

B, H, S, DK = 4, 16, 2048, 64
NCORES = 8
HPC = H * B // NCORES   # 8 heads per core
NPAIR = HPC // 2        # 4 head pairs
QT = 512
NQT = S // QT
KT = 128
NKT = S // KT           # 16
VE = DK + 1
SCALE = 1.0 / float(np.sqrt(DK))

_BF16 = ml_dtypes.bfloat16

_CACHE = {}

# score j-groups per head: head0 uses the 4-bank ring, head1 the 3-bank ring
G0 = [(0, 4), (4, 4), (8, 4), (12, 4)]
G1 = [(0, 3), (3, 3), (6, 3), (9, 3), (12, 3), (15, 1)]


def _build_nc(reps=1):
    pv = True; epi_on = True; mask_on = True
    import concourse.mybir as mybir
    import concourse.tile as tile
    from concourse import bacc
    from contextlib import ExitStack

    dt = mybir.dt
    nc = bacc.Bacc()
    AF = mybir.ActivationFunctionType

    q2 = nc.declare_dram_parameter("q2", [NPAIR, 2 * DK, S], dt.bfloat16, isOutput=False)
    k2 = nc.declare_dram_parameter("k2", [NPAIR, 2 * DK, S], dt.bfloat16, isOutput=False)
    vex = nc.declare_dram_parameter("vex", [S, HPC, VE], dt.bfloat16, isOutput=False)
    maskT = nc.declare_dram_parameter("maskT", [S, S], dt.bfloat16, isOutput=False)
    out = nc.declare_dram_parameter("out", [HPC, S, DK], dt.float32, isOutput=True)

    with tile.TileContext(nc) as tc, ExitStack() as ctx:
        const = ctx.enter_context(tc.tile_pool(name="const", bufs=1))
        maskp = ctx.enter_context(tc.tile_pool(name="maskp", bufs=2))
        qp = ctx.enter_context(tc.tile_pool(name="qp", bufs=2))
        pp = ctx.enter_context(tc.tile_pool(name="pp", bufs=2))
        epi = ctx.enter_context(tc.tile_pool(name="epi", bufs=3))
        psA = ctx.enter_context(tc.tile_pool(name="psA", bufs=1, space="PSUM"))
        psB = ctx.enter_context(tc.tile_pool(name="psB", bufs=1, space="PSUM"))
        pvps = ctx.enter_context(tc.tile_pool(name="pvps", bufs=1, space="PSUM"))

        warm = const.tile([1, 2], dt.float32)
        nc.vector.memset(warm, 0.0)
        nc.scalar.activation(out=warm, in_=warm, func=AF.Exp)

        # K^T head pairs packed [128, pair, s]; pair 0 first for fast start
        k_sb = const.tile([2 * DK, NPAIR, S], dt.bfloat16)
        nc.sync.dma_start(out=k_sb[:, 0, :], in_=k2[0])
        nc.sync.dma_start(
            out=k_sb[:, 1:, :], in_=k2[1:].rearrange("h d s -> d h s")
        )

        v_sb = const.tile([KT, NKT, HPC, VE], dt.bfloat16)
        nc.sync.dma_start(out=v_sb, in_=vex.rearrange("(j p) h e -> p j h e", p=KT))

        # software-pipeline state: pv chains consumed one hp after produced
        prev = None  # (p0_handle, p1_handle, hp, qt)

        def emit_pv_chain(state, chain):
            # chain: 0..3 -> (head hc, half)
            p_handles, hp_prev, qt_prev = state
            hc, half = chain // 2, chain % 2
            h = 2 * hp_prev + hc
            p_sb = p_handles[hc]
            pv = pvps.tile([KT, 2, VE], dt.float32, tag="pv")
            for qb in range(2):
                qcol = (half * 2 + qb) * KT
                for j in range(NKT):
                    nc.tensor.matmul(
                        out=pv[:, qb, :],
                        lhsT=p_sb[:, j, qcol:qcol + KT],
                        rhs=v_sb[:, j, h, :],
                        start=(j == 0),
                        stop=(j == NKT - 1),
                    )
            if not epi_on:
                outf = epi.tile([KT, 2, DK], dt.float32, tag="outf")
                nc.vector.tensor_copy(outf, pv[:, :, 0:DK])
            else:
                rec = epi.tile([KT, 2], dt.float32, tag="rec")
                nc.vector.reciprocal(rec, pv[:, :, DK])
                outf = epi.tile([KT, 2, DK], dt.float32, tag="outf")
                for qb in range(2):
                    nc.vector.tensor_scalar_mul(
                        outf[:, qb, :], pv[:, qb, 0:DK], rec[:, qb:qb + 1]
                    )
            q0 = qt_prev * QT + half * 2 * KT
            nc.sync.dma_start(
                out=out[h, q0:q0 + 2 * KT, :].rearrange("(qb p) d -> p qb d", p=KT),
                in_=outf,
            )

        for _rep in range(reps):
         for qt in range(NQT):
            m_sb = maskp.tile([KT, NKT, QT], dt.bfloat16)
            nc.sync.dma_start(
                out=m_sb,
                in_=maskT[:, qt * QT:(qt + 1) * QT].rearrange(
                    "(j p) q -> p j q", p=KT
                ),
            )
            q_all = qp.tile([2 * DK, NPAIR, QT], dt.bfloat16)
            nc.sync.dma_start(
                out=q_all,
                in_=q2[:, :, qt * QT:(qt + 1) * QT].rearrange("h d q -> d h q"),
            )
            for hp in range(NPAIR):
                p0 = pp.tile([KT, NKT, QT], dt.bfloat16, tag="p0")
                p1 = pp.tile([KT, NKT, QT], dt.bfloat16, tag="p1")
                p_cur = (p0, p1)

                def emit_group(a, j0, nj, ps_pool, width):
                    sc = ps_pool.tile([KT, width], dt.float32, tag="sc")
                    for u in range(nj):
                        j = j0 + u
                        nc.tensor.matmul(
                            out=sc[:, u * QT:(u + 1) * QT],
                            lhsT=k_sb[64 * a:64 * a + 64, hp,
                                      j * KT:(j + 1) * KT],
                            rhs=q_all[64 * a:64 * a + 64, hp, :],
                            start=True,
                            stop=True,
                        )
                    dst = p_cur[a][:, j0:j0 + nj, :].rearrange("p a q -> p (a q)")
                    nc.scalar.activation(
                        out=dst,
                        in_=sc[:, 0:nj * QT],
                        func=AF.Exp,
                        scale=SCALE,
                    )
                    if mask_on:
                        nc.vector.tensor_mul(
                            dst, dst,
                            m_sb[:, j0:j0 + nj, :].rearrange("p a q -> p (a q)"),
                        )

                # interleave head0 (4-bank ring) and head1 (3-bank ring)
                # groups; pv chains of the previous hp between them
                emit_group(0, *G0[0], psA, 4 * QT)
                emit_group(1, *G1[0], psB, 3 * QT)
                emit_group(0, *G0[1], psA, 4 * QT)
                emit_group(1, *G1[1], psB, 3 * QT)
                if pv and prev is not None:
                    emit_pv_chain(prev, 0)
                emit_group(0, *G0[2], psA, 4 * QT)
                emit_group(1, *G1[2], psB, 3 * QT)
                if pv and prev is not None:
                    emit_pv_chain(prev, 1)
                emit_group(0, *G0[3], psA, 4 * QT)
                emit_group(1, *G1[3], psB, 3 * QT)
                if pv and prev is not None:
                    emit_pv_chain(prev, 2)
                emit_group(1, *G1[4], psB, 3 * QT)
                emit_group(1, *G1[5], psB, 3 * QT)
                if pv and prev is not None:
                    emit_pv_chain(prev, 3)

                prev = (p_cur, hp, qt)

        # flush the last head pair
        if pv:
            for chain in range(4):
                emit_pv_chain(prev, chain)
        else:
            dummy = epi.tile([KT, 2, DK], dt.float32, tag="outf")
            nc.vector.tensor_copy(dummy, prev[0][0][:, 0, 0:2 * DK].rearrange("p (a d) -> p a d", a=2))
            nc.sync.dma_start(
                out=out[0, 0:2 * KT, :].rearrange("(qb p) d -> p qb d", p=KT),
                in_=dummy,
            )

    nc.compile()
    return nc


def _get_nc(reps=1, **kw):
    key = ("nc", reps, tuple(sorted(kw.items())))
    if key not in _CACHE:
        _CACHE[key] = _build_nc(reps, **kw)
    return _CACHE[key]


def _prep_core_inputs(q, k, v, m, core):
    b = core // (H // HPC)
    h0 = (core % (H // HPC)) * HPC
    qs = q[b, h0:h0 + HPC].transpose(0, 2, 1).astype(_BF16)   # [8, DK, S]
    ks = k[b, h0:h0 + HPC].transpose(0, 2, 1).astype(_BF16)
    q2 = np.empty((NPAIR, 2 * DK, S), dtype=_BF16)
    k2 = np.empty((NPAIR, 2 * DK, S), dtype=_BF16)
    for hp in range(NPAIR):
        q2[hp, :DK] = qs[2 * hp]
        q2[hp, DK:] = qs[2 * hp + 1]
        k2[hp, :DK] = ks[2 * hp]
        k2[hp, DK:] = ks[2 * hp + 1]
    vex = np.ones((S, HPC, VE), dtype=_BF16)
    vex[:, :, :DK] = v[b, h0:h0 + HPC].transpose(1, 0, 2)
    mT = m[b, 0].T.astype(_BF16)
    return {"q2": q2, "k2": k2, "vex": vex, "maskT": np.ascontiguousarray(mT)}


def kernel(query, key, value, mask):
    from concourse.bass_utils import run_bass_kernel_spmd

    q = np.asarray(query, dtype=np.float32)
    k = np.asarray(key, dtype=np.float32)
    v = np.asarray(value, dtype=np.float32)
    m = np.asarray(mask)

    nc = _get_nc()
    in_maps = [_prep_core_inputs(q, k, v, m, c) for c in range(NCORES)]
    res = run_bass_kernel_spmd(nc, in_maps, list(range(NCORES))).results

    out = np.empty((B, H, S, DK), dtype=np.float32)
    for c in range(NCORES):
        b = c // (H // HPC)
        h0 = (c % (H // HPC)) * HPC
        out[b, h0:h0 + HPC] = res[c]["out"]
    return out
